# revision 19
# baseline (speedup 1.0000x reference)
"""Mixer (token-mix + channel-mix MLP) kernel for 8 TRN2 NeuronCores.

v4d: u-only exchange, receiver-side LN2, fp8 weights.
  Phase 1 (expert-parallel over channels, pair-packed [par,b]
  partitions) computes u = x + tok and LN2 stats (rstd, -mu*rstd per
  (b, c)).  Only u ships, as two half-sized AllToAlls (one per pair of
  chunks) with the 16 bf16 stats columns replicated into each
  destination slot of the payload via a single stride-0-broadcast DMA
  — no separate stats collective.  Send writes go through the
  otherwise-idle gpsimd queue in a d-major layout with 256B-contiguous
  runs.  Weights stream as fp8e4 (ws=64 scale folded into the gelu
  scale and residual STT): token-mix pairs 3-ahead alternating over
  the sync/scalar HWDGE queues, channel-mix pairs prefetched during
  phase 1 on the gpsimd queue into a dedicated SBUF pool so phase 2
  reads no weights from HBM.  x is bf16.  Phase 2 stages u2/stats with
  3-dim DMAs (channel order cg = s*32+h*16+par*8+kk2*4+p folded into
  the cw1/cw2 row/col permutation host-side, inverted in
  assemble_output), normalizes yn2 = u2*rstd + nmr per half with two
  broadcast DVE ops as soon as that half lands, then runs the same
  pair-packed transpose/fc1/gelu/fc2 with the residual add fused into
  the output STT.
"""
import sys
import numpy as np

sys.path.insert(0, "/opt/trn_rl_repo")

import ml_dtypes
import concourse.bass as bass
import concourse.bacc as bacc
import concourse.tile as tile
from concourse import mybir
from concourse.bass_utils import run_bass_kernel_spmd

F32 = mybir.dt.float32
BF16 = mybir.dt.bfloat16
NCORE = 8
B, C, N = 64, 256, 256
CL = C // NCORE   # 32 local channels (phase 1)
NL = N // NCORE   # 32 local patches (phase 2)
EPS = 1e-5
GELU = mybir.ActivationFunctionType.Gelu
IDENT = mybir.ActivationFunctionType.Identity
SQRT = mybir.ActivationFunctionType.Sqrt
MUL = mybir.AluOpType.mult
ADD = mybir.AluOpType.add

CH = 8                 # channels per chunk
NCHUNK = CL // CH      # 4 chunks
PPC = 4                # pairs per chunk
NPAIR = CL // 2        # 16 channel pairs / 16 patch pairs


def build_program(gelu_func=GELU, mmdt=BF16, ws=1.0, skip_b1=True,
                  skip_bc1=True, dbg=False):
    nc = bacc.Bacc("TRN2", target_bir_lowering=False, debug=False,
                   enable_asserts=True, num_devices=NCORE)
    wsi = 1.0 / ws

    x_in = nc.dram_tensor("x_sh", [2, B, PPC * NCHUNK, N], BF16,
                          kind="ExternalInput")
    wt_in = nc.dram_tensor("wt", [NPAIR, 128, 2, 4, N], mmdt,
                           kind="ExternalInput")
    ct_in = nc.dram_tensor("ct", [NPAIR, 128, 2, 4, C], mmdt,
                           kind="ExternalInput")
    id128_in = nc.dram_tensor("id128", [128, 128], BF16,
                              kind="ExternalInput")
    if not skip_b1:
        b1p_in = nc.dram_tensor("b1p", [4, NPAIR, 128], BF16,
                                kind="ExternalInput")
    if not (skip_b1 and skip_bc1):
        msel_in = nc.dram_tensor("msel", [4, 256], BF16,
                                 kind="ExternalInput")
    if not skip_bc1:
        bc1p_in = nc.dram_tensor("bc1p", [4, NPAIR, 128], BF16,
                                 kind="ExternalInput")

    ybuf = nc.dram_tensor("ybuf", [B, NL, C], BF16, kind="ExternalOutput")
    if dbg:
        u_dbg = nc.dram_tensor("u_dbg", [128, 8, 2, 4, 4, 16], BF16,
                               kind="ExternalOutput")
        u2_dbg = nc.dram_tensor("u2_dbg", [128, C, 16], BF16,
                                kind="ExternalOutput")
        st_dbg = nc.dram_tensor("st_dbg", [128, 2, C], BF16,
                                kind="ExternalOutput")
        yn_dbg = nc.dram_tensor("yn_dbg", [128, C], BF16,
                                kind="ExternalOutput")

    with tile.TileContext(nc) as tc:
        with tc.tile_pool(name="const", bufs=1) as const, \
             tc.tile_pool(name="wpool", bufs=6) as wpool, \
             tc.tile_pool(name="cpool", bufs=1) as cpool, \
             tc.tile_pool(name="act", bufs=4) as act, \
             tc.tile_pool(name="scr", bufs=2) as scr, \
             tc.tile_pool(name="dram", bufs=1, space="DRAM") as dram, \
             tc.tile_pool(name="ps", bufs=2, space="PSUM") as ps:

            # exchange buffers: u ships d-major [dest, q=(par,b), a, f]
            # where f = (kk-in-half, p, nl16) is 256B-contiguous
            snd = [dram.tile([NCORE, 128, 272], BF16, name=f"snd{h}")
                   for h in range(2)]
            rcv = [dram.tile([NCORE, 128, 272], BF16, name=f"rcv{h}")
                   for h in range(2)]

            id128 = const.tile([128, 128], BF16)
            nc.sync.dma_start(out=id128[:], in_=id128_in[:])
            if not skip_b1:
                b1p = const.tile([4, NPAIR, 128], BF16)
                nc.sync.dma_start(out=b1p[:], in_=b1p_in[:])
            if not (skip_b1 and skip_bc1):
                msel = const.tile([4, 256], BF16)
                nc.sync.dma_start(out=msel[:], in_=msel_in[:])
            if not skip_bc1:
                bc1p = const.tile([4, NPAIR, 128], BF16)
                nc.sync.dma_start(out=bc1p[:], in_=bc1p_in[:])
            eps128 = const.tile([128, 1], F32)
            nc.vector.memset(eps128[:], EPS)

            # x pair-packed: partition (par, b); par selects (p vs p+4)
            x_sb = [const.tile([128, PPC, N], BF16, name=f"x{k}")
                    for k in range(NCHUNK)]
            for k in range(NCHUNK):
                for par in range(2):
                    nc.sync.dma_start(
                        out=x_sb[k][par * 64:(par + 1) * 64, :, :],
                        in_=x_in[par, :, k * PPC:(k + 1) * PPC, :])

            # u accumulated d-major: [128(par,b), d, a, kk, p, n16]
            u_all = const.tile([128, NCORE, 2, NCHUNK, PPC, 16], BF16)
            # phase-2 staging: [128(a,b), cg, n16] and stats [128, cg, t]
            u2 = const.tile([128, C, 16], BF16)
            stats_b = const.tile([128, C, 2], BF16)

            # LN stats, one column per pair P
            st2 = const.tile([128, NPAIR, 2], F32)   # bn_aggr (mean, var)
            rstd1 = const.tile([128, NPAIR], F32)
            nmr1 = const.tile([128, NPAIR], F32)
            s2 = const.tile([128, NPAIR], F32)
            s2q = const.tile([128, NPAIR], F32)
            mu2 = const.tile([128, NPAIR], F32)
            rstd2 = const.tile([128, NPAIR], F32)
            nmr2 = const.tile([128, NPAIR], F32)
            tv = const.tile([128, NPAIR], F32)
            ts_ = const.tile([128, NPAIR], F32)
            stat1 = const.tile([128, NPAIR, 2], BF16)

            y_pk = [const.tile([128, PPC, C], BF16, name=f"y{k}")
                    for k in range(NCHUNK)]

            def bn1(P):
                k, p = divmod(P, PPC)
                st6 = scr.tile([128, 6], F32, tag="st6")
                nc.vector.bn_stats(out=st6[:], in_=x_sb[k][:, p, :])
                nc.vector.bn_aggr(out=st2[:, P, :], in_=st6[:])

            def ln1_batch(k):
                cs = slice(k * PPC, (k + 1) * PPC)
                nc.scalar.activation(out=ts_[:, cs], in_=st2[:, cs, 1],
                                     func=SQRT, bias=eps128[:], scale=1.0)
                nc.vector.reciprocal(out=rstd1[:, cs], in_=ts_[:, cs])
                nc.vector.scalar_tensor_tensor(
                    out=nmr1[:, cs], in0=st2[:, cs, 0], scalar=-1.0,
                    in1=rstd1[:, cs], op0=MUL, op1=MUL)

            def ln2_batch(k):
                cs = slice(k * PPC, (k + 1) * PPC)
                nc.vector.tensor_scalar_mul(out=mu2[:, cs], in0=s2[:, cs],
                                            scalar1=1.0 / N)
                nc.vector.tensor_scalar_mul(out=tv[:, cs], in0=s2q[:, cs],
                                            scalar1=1.0 / N)
                nc.vector.tensor_mul(out=ts_[:, cs], in0=mu2[:, cs],
                                     in1=mu2[:, cs])
                nc.vector.tensor_sub(out=tv[:, cs], in0=tv[:, cs],
                                     in1=ts_[:, cs])
                nc.scalar.activation(out=ts_[:, cs], in_=tv[:, cs],
                                     func=SQRT, bias=eps128[:], scale=1.0)
                nc.vector.reciprocal(out=rstd2[:, cs], in_=ts_[:, cs])
                nc.vector.scalar_tensor_tensor(
                    out=stat1[:, cs, 1], in0=mu2[:, cs], scalar=-1.0,
                    in1=rstd2[:, cs], op0=MUL, op1=MUL)
                nc.vector.tensor_copy(out=stat1[:, cs, 0],
                                      in_=rstd2[:, cs])

            def stage1_pre(P):
                """xn + pair transpose + z copy for pair P."""
                k, p = divmod(P, PPC)
                xn = act.tile([128, N], BF16, tag="xn")
                nc.scalar.activation(
                    out=xn[:], in_=x_sb[k][:, p, :], func=IDENT,
                    bias=nmr1[:, P:P + 1], scale=rstd1[:, P:P + 1])
                zxp = ps.tile([128, 2, 128], F32, tag="zxp", bufs=3)
                for blk in range(2):
                    nc.tensor.matmul(
                        zxp[:, blk, :],
                        xn[:, blk * 128:(blk + 1) * 128],
                        id128[:], start=True, stop=True)
                z_sb = act.tile([128, 2, 128], BF16, tag="z")
                nc.vector.tensor_copy(out=z_sb[:], in_=zxp[:])
                return z_sb

            def mix_pair(P, z_sb, wp, bp, skip_b):
                """fc1 + gelu + fc2 for both groups of pair P."""
                hpre = ps.tile([128, 2, 2, 64], F32, tag="hpre")
                if not skip_b:
                    nc.tensor.matmul(
                        hpre[:].rearrange("p a b c -> p (a b c)"),
                        bp[:, P, :], msel[:],
                        start=True, stop=False, skip_group_check=True)
                for par in range(2):
                    for mb in range(2):
                        for nb in range(2):
                            nc.tensor.matmul(
                                hpre[:, mb, par, :],
                                wp[:, par, nb, mb * 128:(mb + 1) * 128],
                                z_sb[:, nb, par * 64:(par + 1) * 64],
                                start=(skip_b and nb == 0), stop=(nb == 1),
                                skip_group_check=True)
                hs = act.tile([128, 2, 2, 64], BF16, tag="h")
                nc.scalar.activation(out=hs[:], in_=hpre[:], func=gelu_func,
                                     scale=wsi)

                tok = ps.tile([128, 256], F32, tag="tok")
                for par in range(2):
                    for mb in range(2):
                        nc.tensor.matmul(
                            tok[par * 64:(par + 1) * 64, :],
                            hs[:, mb, par, :],
                            wp[:, par, 2 + mb, :],
                            start=(mb == 0), stop=(mb == 1),
                            skip_group_check=True)
                return tok

            def stage1_post(P, tok):
                k, p = divmod(P, PPC)
                tok3 = tok.rearrange("q (d a n) -> q d a n", d=NCORE, a=2)
                x3 = x_sb[k][:, p, :].rearrange("q (d a n) -> q d a n",
                                                d=NCORE, a=2)
                uv = u_all[:, :, :, k, p, :]
                nc.vector.scalar_tensor_tensor(
                    out=uv, in0=tok3, scalar=wsi,
                    in1=x3, op0=MUL, op1=ADD, accum_out=s2[:, P:P + 1])
                sqs = scr.tile([128, NCORE, 2, 16], BF16, tag="sqs")
                nc.vector.scalar_tensor_tensor(
                    out=sqs[:], in0=uv, scalar=1.0, in1=uv,
                    op0=MUL, op1=MUL, accum_out=s2q[:, P:P + 1])

            # phase-2 weights: dedicated pool, prefetched during phase 1
            cp_pre = []
            for Q in range(NPAIR):
                cpq = cpool.tile([128, 2, 4, C], mmdt, name=f"cp{Q}")
                cp_pre.append(cpq)

            # phase-1 weight stream, 3 pairs ahead
            wps = [None] * NPAIR

            def load_wt(P):
                wps[P] = wpool.tile([128, 2, 4, N], mmdt, tag="w",
                                    name=f"w{P}")
                eng = nc.sync if P % 2 == 0 else nc.scalar
                eng.dma_start(out=wps[P][:], in_=wt_in[P])

            # ---------------- phase 1 ----------------
            for P in range(3):
                load_wt(P)
            for P in range(PPC + 2):
                bn1(P)
            ln1_batch(0)
            z_cur = stage1_pre(0)
            z_nxt = stage1_pre(1)
            for P in range(NPAIR):
                k, p = divmod(P, PPC)
                if P + 3 < NPAIR:
                    load_wt(P + 3)
                nc.gpsimd.dma_start(out=cp_pre[P][:], in_=ct_in[P])

                if P + PPC + 2 < NPAIR:
                    bn1(P + PPC + 2)
                if p == 1 and k + 1 < NCHUNK:
                    ln1_batch(k + 1)

                z_n2 = stage1_pre(P + 2) if P + 2 < NPAIR else None
                tok = mix_pair(P, z_cur, wps[P],
                               b1p if not skip_b1 else None, skip_b1)
                stage1_post(P, tok)
                wps[P] = None
                z_cur, z_nxt = z_nxt, z_n2

                if p == PPC - 1:
                    ln2_batch(k)
                    if k % 2 == 1:
                        h = k // 2
                        for a in range(2):
                            nc.gpsimd.dma_start(
                                out=snd[h][:, :, a * 128:(a + 1) * 128]
                                .rearrange("d q f -> q d f"),
                                in_=u_all[:, :, a, 2 * h:2 * h + 2, :, :]
                                .rearrange("q d kk p n -> q d (kk p n)"))
                        stv = stat1[:, 8 * h:8 * h + 8, :]
                        apl = [list(x) for x in stv.ap]
                        stbc = bass.AP(
                            tensor=stv.tensor, offset=stv.offset,
                            ap=[apl[0], [0, NCORE]] + apl[1:])
                        nc.gpsimd.dma_start(
                            out=snd[h][:, :, 256:272]
                            .rearrange("d q (e t) -> q d e t", t=2),
                            in_=stbc)
                        nc.gpsimd.collective_compute(
                            "AllToAll", mybir.AluOpType.bypass,
                            replica_groups=[list(range(NCORE))],
                            ins=[snd[h].opt()], outs=[rcv[h].opt()])

            if dbg:
                nc.scalar.dma_start(out=u_dbg[:], in_=u_all[:])

            # ---------------- phase 2 staging ----------------
            # u2[(a,b), cg, n] with cg = s*32 + h*16 + par*8 + kk2*4 + p
            for h in range(2):
                for a in range(2):
                    for par in range(2):
                        nc.sync.dma_start(
                            out=u2[a * 64:(a + 1) * 64]
                            .rearrange("b (s hh pp f) n -> b s hh pp (f n)",
                                       s=NCORE, hh=2, pp=2)[:, :, h, par],
                            in_=rcv[h].rearrange(
                                "s (pp b) f -> pp b s f",
                                pp=2)[par, :, :, a * 128:(a + 1) * 128])
                        nc.scalar.dma_start(
                            out=stats_b[a * 64:(a + 1) * 64]
                            .rearrange("b (s hh pp e) t -> b s hh pp (e t)",
                                       s=NCORE, hh=2, pp=2)[:, :, h, par],
                            in_=rcv[h].rearrange(
                                "s (pp b) f -> pp b s f",
                                pp=2)[par, :, :, 256:272])
            if dbg:
                nc.scalar.dma_start(out=u2_dbg[:], in_=u2[:])
                nc.scalar.dma_start(out=st_dbg[:], in_=stats_b[:])

            yn2 = const.tile([128, C, 16], BF16)
            rstd_bc, _ = bass.broadcast_tensor_aps(
                stats_b[:, :, 0:1], u2[:])
            nmr_bc, _ = bass.broadcast_tensor_aps(
                stats_b[:, :, 1:2], u2[:])
            nc.vector.scalar_tensor_tensor(
                out=yn2[:], in0=u2[:], scalar=1.0,
                in1=rstd_bc, op0=MUL, op1=MUL)
            nc.vector.tensor_add(out=yn2[:], in0=yn2[:], in1=nmr_bc)

            def stage2_pre(Q):
                z2p = ps.tile([128, 2, 128], F32, tag="zxp", bufs=3)
                for cb in range(2):
                    nc.tensor.matmul(
                        z2p[:, cb, :],
                        yn2[:, cb * 128:(cb + 1) * 128, Q],
                        id128[:], start=True, stop=True)
                z2 = act.tile([128, 2, 128], BF16, tag="z")
                nc.vector.tensor_copy(out=z2[:], in_=z2p[:])
                return z2

            if dbg:
                nc.scalar.dma_start(out=yn_dbg[:], in_=yn2[:, :, 0])
            z2_cur = stage2_pre(0)
            z2_nxt = stage2_pre(1)
            for Q in range(NPAIR):
                k2, q2 = divmod(Q, PPC)
                z2_n2 = (stage2_pre(Q + 2)
                         if Q + 2 < NPAIR else None)
                ch_ps = mix_pair(Q, z2_cur, cp_pre[Q],
                                 bc1p if not skip_bc1 else None, skip_bc1)
                nc.vector.scalar_tensor_tensor(
                    out=y_pk[k2][:, q2, :], in0=ch_ps, scalar=wsi,
                    in1=u2[:, :, Q], op0=MUL, op1=ADD)
                z2_cur, z2_nxt = z2_nxt, z2_n2
                if q2 == PPC - 1:
                    for a in range(2):
                        nc.scalar.dma_start(
                            out=ybuf[:, a * 16 + k2 * PPC:
                                     a * 16 + (k2 + 1) * PPC, :],
                            in_=y_pk[k2][a * 64:(a + 1) * 64])

    nc.finalize()
    return nc


def _cperm():
    """cg -> c_global: cg = s*32 + h*16 + par*8 + kk2*4 + p maps to
    channel s*32 + (2h+kk2)*8 + par*4 + p."""
    perm = np.empty(C, np.int64)
    for s in range(NCORE):
        for h in range(2):
            for par in range(2):
                for kk2 in range(2):
                    for p in range(4):
                        cg = s * 32 + h * 16 + par * 8 + kk2 * 4 + p
                        perm[cg] = s * 32 + (2 * h + kk2) * 8 + par * 4 + p
    return perm


CPERM = _cperm()


def prep_inputs(x, g1, be1, g2, be2, tw1, tb1, tw2, tb2, cw1, cb1, cw2, cb2,
                mmdt_np=ml_dtypes.bfloat16, ws=1.0):
    """Host-side sharding + weight folding. Returns in_maps for the 8 cores."""
    f = np.float32
    x = np.asarray(x, f)
    g1, be1, g2, be2 = (np.asarray(a, f) for a in (g1, be1, g2, be2))
    tw1, tb1, tw2, tb2 = (np.asarray(a, f) for a in (tw1, tb1, tw2, tb2))
    cw1, cb1, cw2, cb2 = (np.asarray(a, f) for a in (cw1, cb1, cw2, cb2))

    def wcast(a):
        a = a * ws
        if mmdt_np is not ml_dtypes.bfloat16:
            a = np.clip(a, -240.0, 240.0)
        return a.astype(mmdt_np)

    w1t = (tw1 * g1[None, None, :]).transpose(0, 2, 1)            # [C, N, M]
    bias1 = (tb1 + np.einsum('n,cmn->cm', be1, tw1)) * ws         # [C, M]
    w2t = tw2.transpose(0, 2, 1)                                  # [c, m, k]
    t1r = w1t.reshape(C, 2, 128, N)
    t2r = w2t.reshape(C, 2, 128, N)
    wt = np.ascontiguousarray(
        np.stack([t1r[:, 0], t1r[:, 1], t2r[:, 0], t2r[:, 1]],
                 axis=2))                                         # [C,128,4,N]

    # channel-mix: contraction rows (c) and output rows (k) in cg order
    c1t = (cw1 * g2[:, None, None]).transpose(0, 2, 1)[:, CPERM, :]
    biasc1 = (cb1 + be2[:, None] * cw1.sum(axis=2)) * ws          # [N, O]
    c2t = cw2.transpose(0, 2, 1)[:, :, CPERM]                     # [n, o, kg]
    c1r = c1t.reshape(N, 2, 128, C)
    c2r = c2t.reshape(N, 2, 128, C)
    ct = np.ascontiguousarray(
        np.stack([c1r[:, 0], c1r[:, 1], c2r[:, 0], c2r[:, 1]],
                 axis=2))                                         # [N,128,4,C]

    id128 = np.eye(128, dtype=f).astype(ml_dtypes.bfloat16)
    msel = np.zeros((4, 2, 2, 64), f)
    for mb in range(2):
        for par in range(2):
            msel[mb * 2 + par, mb, par, :] = 1.0
    msel = msel.reshape(4, 256).astype(ml_dtypes.bfloat16)

    # channel pair order within a core: chunk k has channels k*8+ci,
    # pairs are (ci, ci+4); patch pairs are (nl, nl+16)
    cpair0 = np.array([k * CH + p for k in range(NCHUNK)
                       for p in range(PPC)])                      # 16
    npair0 = np.arange(16)

    def pair_pack(wfull, p0, off):   # [G,128,4,X] -> [G/2,128,2,4,X]
        a = wfull[p0]
        b = wfull[p0 + off]
        return np.ascontiguousarray(np.stack([a, b], axis=2))

    def bias_pair(bm, p0, off):      # [G,256] -> [4, G/2, 128] (mb*2+par)
        out = np.empty((4, len(p0), 128), f)
        for mb in range(2):
            for par in range(2):
                out[mb * 2 + par] = bm[p0 + par * off,
                                       mb * 128:(mb + 1) * 128]
        return np.ascontiguousarray(out).astype(ml_dtypes.bfloat16)

    in_maps = []
    for m in range(NCORE):
        cs = slice(m * CL, (m + 1) * CL)
        ns = slice(m * NL, (m + 1) * NL)
        xl = x[:, cs, :]                                          # [B,CL,N]
        # pair-packed x: [2(par), B, 16(chunk-major pairs), N]
        xp = np.stack([xl[:, cpair0, :], xl[:, cpair0 + 4, :]], axis=0)
        wtl = wcast(wt[cs])
        ctl = wcast(ct[ns])
        d = {
            "x_sh": np.ascontiguousarray(xp).astype(ml_dtypes.bfloat16),
            "wt": pair_pack(wtl, cpair0, 4),
            "ct": pair_pack(ctl, npair0, 16),
            "id128": id128,
        }
        if np.any(bias1):
            d["b1p"] = bias_pair(bias1[cs], cpair0, 4)
            d["msel"] = msel
        if np.any(biasc1):
            d["bc1p"] = bias_pair(biasc1[ns], npair0, 16)
            d["msel"] = msel
        in_maps.append(d)
    return in_maps


def assemble_output(results):
    """results: per-core dicts with 'ybuf' [B, NL, C-in-cg-order]."""
    y = np.empty((B, C, N), np.float32)
    for m in range(NCORE):
        blk = results[m]["ybuf"].astype(np.float32)   # [B, 32, 256]
        y[:, CPERM, m * NL:(m + 1) * NL] = blk.transpose(0, 2, 1)
    return y


_PROGRAMS = {}

# weight dtype config: (mybir dtype, numpy dtype, weight scale)
USE_FP8 = True
_W_CFG = ((mybir.dt.float8e4, ml_dtypes.float8_e4m3, 64.0) if USE_FP8
          else (BF16, ml_dtypes.bfloat16, 1.0))


def get_program(skip_b2=True, skip_bc2=True, skip_b1=True, skip_bc1=True):
    key = (skip_b1, skip_bc1, USE_FP8)
    if key not in _PROGRAMS:
        _PROGRAMS[key] = build_program(
            mmdt=_W_CFG[0], ws=_W_CFG[2],
            skip_b1=skip_b1, skip_bc1=skip_bc1)
    return _PROGRAMS[key]


def kernel(**inputs):
    skip_b1 = not (np.any(np.asarray(inputs["tb1"]))
                   or np.any(np.asarray(inputs["be1"])))
    skip_bc1 = not (np.any(np.asarray(inputs["cb1"]))
                    or np.any(np.asarray(inputs["be2"])))
    prog = get_program(True, True, skip_b1, skip_bc1)
    in_maps = prep_inputs(**inputs, mmdt_np=_W_CFG[1], ws=_W_CFG[2])
    res = run_bass_kernel_spmd(prog, in_maps, list(range(NCORE)))
    out = assemble_output(res.results)
    # tb2 / cb2 are added host-side (zero for the target problem, but
    # keep the general contract correct)
    tb2 = np.asarray(inputs["tb2"], np.float32)
    cb2 = np.asarray(inputs["cb2"], np.float32)
    if np.any(tb2):
        out = out + tb2[None]
    if np.any(cb2):
        out = out + cb2.T[None]
    return out


if __name__ == "__main__":
    from scipy.special import erf

    rng = np.random.RandomState(0)
    s = 0.02
    inputs = dict(
        x=rng.randn(B, C, N).astype(np.float32),
        g1=np.ones(N, np.float32), be1=np.zeros(N, np.float32),
        g2=np.ones(N, np.float32), be2=np.zeros(N, np.float32),
        tw1=(rng.randn(C, N, N) * s).astype(np.float32),
        tb1=np.zeros((C, N), np.float32),
        tw2=(rng.randn(C, N, N) * s).astype(np.float32),
        tb2=np.zeros((C, N), np.float32),
        cw1=(rng.randn(N, C, C) * s).astype(np.float32),
        cb1=np.zeros((N, C), np.float32),
        cw2=(rng.randn(N, C, C) * s).astype(np.float32),
        cb2=np.zeros((N, C), np.float32),
    )

    def np_ref(x, g1, be1, g2, be2, tw1, tb1, tw2, tb2, cw1, cb1, cw2, cb2):
        def ln(z, g, b):
            mu = z.mean(-1, keepdims=True)
            var = z.var(-1, keepdims=True)
            return (z - mu) / np.sqrt(var + EPS) * g + b
        def gelu(v):
            return v * 0.5 * (1 + erf(v / np.sqrt(2.0)))
        xn = ln(x, g1, be1)
        h = gelu(np.einsum('bcn,cmn->bcm', xn, tw1) + tb1[None])
        tok = np.einsum('bcm,ckm->bck', h, tw2) + tb2[None]
        x = x + tok
        yn = ln(x, g2, be2)
        h2 = gelu(np.einsum('bcn,noc->bon', yn, cw1) + cb1.T[None])
        ch = np.einsum('bon,nko->bkn', h2, cw2) + cb2.T[None]
        return x + ch

    exp = np_ref(**{k: v.astype(np.float64) for k, v in inputs.items()})
    got = kernel(**inputs)
    err = np.abs(got - exp)
    rel = err.max() / np.abs(exp).max()
    print(f"abs err: {err.max():.3e}  rel(absmax): {rel:.3e}")
